# revision 2
# baseline (speedup 1.0000x reference)
"""Trainium2 Bass kernel for nn_NodeEncoder (per-type Linear over interleaved node types).

Problem: x [800000, 128] f32, W [8, 256, 128], b [8, 256].
Node n has type k = n % 8; y[n] = (W[k] * mask_k) @ x[n] + b[k], y [800000, 256].

Strategy (8 cores, data-parallel over graphs, weights replicated):
  - Each core gets 100000 consecutive nodes (12500 graphs), padded to
    100352 = 49 super-tiles of 2048 nodes (256 graphs).
  - A contiguous 1 MiB DMA of 2048 nodes lands in SBUF [128, 2048] where
    partition p holds nodes 16p..16p+15.  Free-slice j (cols 128j..128j+128)
    is then a [node, dim] matrix of 128 nodes that are ALL of type j%8
    (node = base + 16p + j, and 16p = 0 mod 8).
  - Per slice: PE-transpose (identity matmul) -> PSUM [d, node] -> DVE copy
    to SBUF; bias matmul (ones[1,128].T @ b_k[1,256], start=True) + compute
    matmul (xt[d,128].T @ WT_k[d,256], start=False) accumulate in PSUM;
    ScalarE copies PSUM -> SBUF out slice.
  - Out SBUF [128, 4096] maps linearly to 2048 output rows -> one contiguous
    2 MiB DMA out.  All DMAs are fully contiguous.
W is pre-masked + pre-transposed on host (it is tiny: 1 MB).
"""

import os
import sys

import numpy as np

for _p in ("/root/.axon_site", "/root/.axon_site/_ro/trn_rl_repo", "/root/.axon_site/_ro/pypackages"):
    if os.path.isdir(_p) and _p not in sys.path:
        sys.path.append(_p)

import concourse.bass as bass
import concourse.mybir as mybir
import concourse.tile as tile
from concourse import bacc
from concourse.bass_utils import run_bass_kernel_spmd
from concourse.masks import make_identity

N_TYPES = 8
MAX_DIM = 128
FEAT = 256
N_GRAPHS = 100000
NODE_DIMS = np.array([16, 32, 64, 128, 64, 32, 16, 128], dtype=np.int32)

N_CORES = 8
NODES_PER_CORE = N_GRAPHS * N_TYPES // N_CORES  # 100000
SUPER_NODES = 2048          # nodes per super-tile (256 graphs)
N_SUPER = 49                # super-tiles per core
PAD_NODES = SUPER_NODES * N_SUPER  # 100352
SLICES = SUPER_NODES // 128  # 16 slices of 128 nodes per super-tile

_F32 = mybir.dt.float32
_nc_cache = {}


def _build_nc():
    if "nc" in _nc_cache:
        return _nc_cache["nc"]
    nc = bacc.Bacc("TRN2", target_bir_lowering=False, debug=False)
    x = nc.dram_tensor("x", [N_SUPER, 128, SUPER_NODES], _F32, kind="ExternalInput").ap()
    wt = nc.dram_tensor("wt", [128, N_TYPES * FEAT], _F32, kind="ExternalInput").ap()
    bvec = nc.dram_tensor("bvec", [1, N_TYPES * FEAT], _F32, kind="ExternalInput").ap()
    y = nc.dram_tensor("y", [N_SUPER, 128, SLICES * FEAT], _F32, kind="ExternalOutput").ap()

    with tile.TileContext(nc) as tc:
        with (
            tc.tile_pool(name="const", bufs=1) as const,
            tc.tile_pool(name="xin", bufs=3) as xin_pool,
            tc.tile_pool(name="xt", bufs=4) as xt_pool,
            tc.tile_pool(name="outsb", bufs=3) as out_pool,
            tc.tile_pool(name="ps_t", bufs=3, space="PSUM") as ps_t,
            tc.tile_pool(name="ps_o", bufs=4, space="PSUM") as ps_o,
        ):
            wt_sb = const.tile([128, N_TYPES * FEAT], _F32)
            nc.sync.dma_start(wt_sb[:], wt[:])
            b_sb = const.tile([1, N_TYPES * FEAT], _F32)
            nc.sync.dma_start(b_sb[:], bvec[:])
            ones = const.tile([1, 128], _F32)
            nc.gpsimd.memset(ones[:], 1.0)
            ident = const.tile([128, 128], _F32)
            make_identity(nc, ident[:])

            for s in range(N_SUPER):
                xs = xin_pool.tile([128, SUPER_NODES], _F32)
                nc.sync.dma_start(xs[:], x[s])
                out_sb = out_pool.tile([128, SLICES * FEAT], _F32)
                for j in range(SLICES):
                    k = j % N_TYPES
                    pxt = ps_t.tile([128, 128], _F32)
                    nc.tensor.transpose(pxt[:], xs[:, j * 128:(j + 1) * 128], ident[:])
                    xt = xt_pool.tile([128, 128], _F32)
                    nc.vector.tensor_copy(xt[:], pxt[:])
                    po = ps_o.tile([128, FEAT], _F32)
                    nc.tensor.matmul(
                        po[:], ones[:], b_sb[:, k * FEAT:(k + 1) * FEAT],
                        start=True, stop=False,
                    )
                    nc.tensor.matmul(
                        po[:], xt[:], wt_sb[:, k * FEAT:(k + 1) * FEAT],
                        start=False, stop=True,
                    )
                    nc.scalar.copy(out_sb[:, j * FEAT:(j + 1) * FEAT], po[:])
                nc.scalar.dma_start(y[s], out_sb[:])

    nc.finalize()
    _nc_cache["nc"] = nc
    return nc


def _prep_weights(W, b):
    mask = (np.arange(MAX_DIM)[None, None, :] < NODE_DIMS[:, None, None])
    W_eff = np.where(mask, W, 0).astype(np.float32)  # [T, F, D]
    wt = np.ascontiguousarray(W_eff.transpose(2, 0, 1).reshape(MAX_DIM, N_TYPES * FEAT))
    bvec = np.ascontiguousarray(b.astype(np.float32).reshape(1, N_TYPES * FEAT))
    return wt, bvec


def run(x, W, b, trace=False):
    nc = _build_nc()
    wt, bvec = _prep_weights(W, b)
    in_maps = []
    for c in range(N_CORES):
        xc = np.zeros((PAD_NODES, MAX_DIM), dtype=np.float32)
        xc[:NODES_PER_CORE] = x[c * NODES_PER_CORE:(c + 1) * NODES_PER_CORE]
        in_maps.append({
            "x": xc.reshape(N_SUPER, 128, SUPER_NODES),
            "wt": wt,
            "bvec": bvec,
        })
    res = run_bass_kernel_spmd(nc, in_maps, list(range(N_CORES)), trace=trace)
    y = np.empty((N_GRAPHS * N_TYPES, FEAT), dtype=np.float32)
    for c in range(N_CORES):
        yc = np.asarray(res.results[c]["y"]).reshape(PAD_NODES, FEAT)
        y[c * NODES_PER_CORE:(c + 1) * NODES_PER_CORE] = yc[:NODES_PER_CORE]
    return y, res


def kernel(**inputs):
    y, _ = run(inputs["x"], inputs["W"], inputs["b"])
    return y


if __name__ == "__main__":
    rng = np.random.default_rng(0)
    x = rng.standard_normal((N_GRAPHS * N_TYPES, MAX_DIM), dtype=np.float32)
    W = (rng.standard_normal((N_TYPES, FEAT, MAX_DIM), dtype=np.float32) * 0.05)
    b = (rng.standard_normal((N_TYPES, FEAT), dtype=np.float32) * 0.05)
    y, res = run(x, W, b)
    # numpy reference on a few rows
    mask = (np.arange(MAX_DIM)[None, None, :] < NODE_DIMS[:, None, None])
    W_eff = np.where(mask, W, 0).astype(np.float32)
    idx = rng.integers(0, N_GRAPHS * N_TYPES, 64)
    exp = np.stack([W_eff[n % 8] @ x[n] + b[n % 8] for n in idx])
    act = y[idx]
    err = np.abs(act - exp).max() / (np.abs(exp).max() + 1e-30)
    print("spot-check rel err:", err)


# revision 10
# speedup vs baseline: 5.1362x; 5.1362x over previous
"""Trainium2 Bass kernel for nn_NodeEncoder (per-type Linear over interleaved node types).

Problem: x [800000, 128] f32, W [8, 256, 128], b [8, 256].
Node n has type k = n % 8; y[n] = (W[k] * mask_k) @ x[n] + b[k], y [800000, 256].

Strategy (8 cores, data-parallel over graphs, weights replicated):
  - Each core gets 100000 consecutive nodes (12500 graphs), padded to
    100352 = 49 super-tiles of 2048 nodes (256 graphs).
  - x is cast to fp16 on the host (round-to-nearest, ~2.4e-4 rel err per
    element; PE multiplies at FP22 so no further loss) which also halves
    the x HBM read traffic.  A contiguous 512 KiB DMA of 2048 nodes lands
    in SBUF [128, 2048] fp16 where partition p holds nodes 16p..16p+15.
    Free-slice j (cols 128j..128j+128) is a [node, dim] matrix of 128
    nodes ALL of type j%8 (node = base + 16p + j, 16p = 0 mod 8).
  - Per slice: PE-transpose (fp16 identity matmul, restricted to the
    type's true input dim) -> PSUM -> DVE copy to SBUF.  For types with
    dim < 128 the host wrote 1.0 into x column `dim` (masked region), so
    the transpose yields [x.T; ones] and the bias rides as contraction
    row `dim` of the weight tile (y = [x,1] @ [W^T; b]).  For the two
    dim-128 types the (exact fp32) bias is added by GpSimd after
    eviction.
  - fp16 matmul accumulates in fp32 PSUM; pairs of slices share one PSUM
    bank [128, 512] so ScalarE evicts two slices per ACTIVATE into the
    fp32 out tile [128, 4096], which maps linearly to 2048 output rows ->
    one contiguous 2 MiB DMA out.  All DMAs are fully contiguous.
W is pre-masked + pre-transposed on host (it is tiny: 1 MB).
"""

import os
import sys

import numpy as np

for _p in ("/root/.axon_site", "/root/.axon_site/_ro/trn_rl_repo", "/root/.axon_site/_ro/pypackages"):
    if os.path.isdir(_p) and _p not in sys.path:
        sys.path.append(_p)

import concourse.bass as bass
import concourse.mybir as mybir
import concourse.tile as tile
from concourse import bacc
from concourse.bass_utils import run_bass_kernel_spmd

N_TYPES = 8
MAX_DIM = 128
FEAT = 256
N_GRAPHS = 100000
NODE_DIMS = np.array([16, 32, 64, 128, 64, 32, 16, 128], dtype=np.int32)

N_CORES = 8
NODES_PER_CORE = N_GRAPHS * N_TYPES // N_CORES  # 100000
SUPER_NODES = 2048          # nodes per super-tile (256 graphs)
N_SUPER = 49                # super-tiles per core
PAD_NODES = SUPER_NODES * N_SUPER  # 100352
SLICES = SUPER_NODES // 128  # 16 slices of 128 nodes per super-tile

_F32 = mybir.dt.float32
_F16 = mybir.dt.float16
BIG_TYPES = [3, 7]          # types with dim == 128 (bias added on GpSimd)
_nc_cache = {}


def _build_nc():
    if "nc" in _nc_cache:
        return _nc_cache["nc"]
    nc = bacc.Bacc("TRN2", target_bir_lowering=False, debug=False)
    x = nc.dram_tensor("x", [N_SUPER, 128, SUPER_NODES], _F16, kind="ExternalInput").ap()
    wtb = nc.dram_tensor("wtb", [128, N_TYPES * FEAT], _F16, kind="ExternalInput").ap()
    bias_full = nc.dram_tensor("bias_full", [128, len(BIG_TYPES) * FEAT], _F32, kind="ExternalInput").ap()
    ident_in = nc.dram_tensor("ident", [128, 128], _F16, kind="ExternalInput").ap()
    y = nc.dram_tensor("y", [N_SUPER, 128, SLICES * FEAT], _F32, kind="ExternalOutput").ap()

    with tile.TileContext(nc) as tc:
        with (
            tc.tile_pool(name="const", bufs=1) as const,
            tc.tile_pool(name="xin", bufs=4) as xin_pool,
            tc.tile_pool(name="xt", bufs=6) as xt_pool,
            tc.tile_pool(name="outsb", bufs=4) as out_pool,
            tc.tile_pool(name="ps_t", bufs=3, space="PSUM") as ps_t,
            tc.tile_pool(name="ps_o", bufs=4, space="PSUM") as ps_o,
        ):
            wtb_sb = const.tile([128, N_TYPES * FEAT], _F16)
            nc.sync.dma_start(wtb_sb[:], wtb[:])
            bias_sb = const.tile([128, len(BIG_TYPES) * FEAT], _F32)
            nc.sync.dma_start(bias_sb[:], bias_full[:])
            ident = const.tile([128, 128], _F16)
            nc.sync.dma_start(ident[:], ident_in[:])

            for s in range(N_SUPER):
                xs = xin_pool.tile([128, SUPER_NODES], _F16)
                nc.sync.dma_start(xs[:], x[s])
                out_sb = out_pool.tile([128, SLICES * FEAT], _F32)
                for jp in range(SLICES // 2):
                    po = ps_o.tile([128, 2 * FEAT], _F32)
                    for half in range(2):
                        j = 2 * jp + half
                        k = j % N_TYPES
                        dim = int(NODE_DIMS[k])
                        # host wrote 1.0 into x column `dim` for dim<128, so
                        # transposing dim+1 columns yields [x.T; ones] and the
                        # bias rides as contraction row `dim` of wtb.
                        kk = dim + 1 if dim < 128 else 128
                        pxt = ps_t.tile([128, 128], _F16)
                        xt = xt_pool.tile([128, 128], _F16)
                        nc.tensor.transpose(pxt[0:kk, :], xs[:, j * 128:j * 128 + kk], ident[:])
                        nc.vector.tensor_copy(xt[0:kk, :], pxt[0:kk, :])
                        nc.tensor.matmul(
                            po[:, half * FEAT:(half + 1) * FEAT],
                            xt[0:kk, :], wtb_sb[0:kk, k * FEAT:(k + 1) * FEAT],
                            start=True, stop=True,
                        )
                    osl = out_sb[:, 2 * jp * FEAT:(2 * jp + 2) * FEAT]
                    nc.scalar.copy(osl, po[:])
                    for half in range(2):
                        j = 2 * jp + half
                        k = j % N_TYPES
                        if k in BIG_TYPES:
                            t = BIG_TYPES.index(k)
                            nc.gpsimd.tensor_add(
                                out_sb[:, j * FEAT:(j + 1) * FEAT],
                                out_sb[:, j * FEAT:(j + 1) * FEAT],
                                bias_sb[:, t * FEAT:(t + 1) * FEAT],
                            )
                nc.scalar.dma_start(y[s], out_sb[:])

    nc.finalize()
    _nc_cache["nc"] = nc
    return nc


def _prep_weights(W, b):
    mask = (np.arange(MAX_DIM)[None, None, :] < NODE_DIMS[:, None, None])
    W_eff = np.where(mask, W, 0).astype(np.float32)  # [T, F, D]
    # wtb[d, k*256+f]: rows 0..dim_k-1 = W_eff[k].T; row dim_k = b[k] (small types)
    wtb = np.zeros((MAX_DIM, N_TYPES * FEAT), dtype=np.float32)
    for k in range(N_TYPES):
        dim = int(NODE_DIMS[k])
        wtb[:dim, k * FEAT:(k + 1) * FEAT] = W_eff[k, :, :dim].T
        if dim < MAX_DIM:
            wtb[dim, k * FEAT:(k + 1) * FEAT] = b[k]
    bias_full = np.ascontiguousarray(
        np.broadcast_to(
            np.concatenate([b[k] for k in BIG_TYPES]).astype(np.float32)[None, :],
            (128, len(BIG_TYPES) * FEAT),
        )
    )
    return wtb.astype(np.float16), bias_full


def run(x, W, b, trace=False):
    nc = _build_nc()
    wtb, bias_full = _prep_weights(W, b)
    ident = np.eye(128, dtype=np.float16)
    in_maps = []
    for c in range(N_CORES):
        xc = np.zeros((PAD_NODES, MAX_DIM), dtype=np.float32)
        xc[:NODES_PER_CORE] = x[c * NODES_PER_CORE:(c + 1) * NODES_PER_CORE]
        for k in range(N_TYPES):
            dim = int(NODE_DIMS[k])
            if dim < MAX_DIM:
                xc[k::N_TYPES, dim] = 1.0  # ones-row for the folded bias
        in_maps.append({
            "x": xc.astype(np.float16).reshape(N_SUPER, 128, SUPER_NODES),
            "wtb": wtb,
            "bias_full": bias_full,
            "ident": ident,
        })
    res = run_bass_kernel_spmd(nc, in_maps, list(range(N_CORES)), trace=trace)
    y = np.empty((N_GRAPHS * N_TYPES, FEAT), dtype=np.float32)
    for c in range(N_CORES):
        yc = np.asarray(res.results[c]["y"]).reshape(PAD_NODES, FEAT)
        y[c * NODES_PER_CORE:(c + 1) * NODES_PER_CORE] = yc[:NODES_PER_CORE]
    return y, res


def kernel(**inputs):
    y, _ = run(inputs["x"], inputs["W"], inputs["b"])
    return y


if __name__ == "__main__":
    rng = np.random.default_rng(0)
    x = rng.standard_normal((N_GRAPHS * N_TYPES, MAX_DIM), dtype=np.float32)
    W = (rng.standard_normal((N_TYPES, FEAT, MAX_DIM), dtype=np.float32) * 0.05)
    b = (rng.standard_normal((N_TYPES, FEAT), dtype=np.float32) * 0.05)
    y, res = run(x, W, b)
    mask = (np.arange(MAX_DIM)[None, None, :] < NODE_DIMS[:, None, None])
    W_eff = np.where(mask, W, 0).astype(np.float32)
    idx = rng.integers(0, N_GRAPHS * N_TYPES, 256)
    exp = np.stack([W_eff[n % 8] @ x[n] + b[n % 8] for n in idx])
    act = y[idx]
    err = np.abs(act - exp).max() / (np.abs(exp).max() + 1e-30)
    print("spot-check rel err:", err)
